# revision 36
# baseline (speedup 1.0000x reference)
"""Trainium2 Bass kernel for nn_Attend (decomposable attention).

Computation (reference):
    f_A = relu(relu(A @ W1 + b1) @ W2 + b2)      [b, m, h]
    f_B = relu(relu(B @ W1 + b1) @ W2 + b2)      [b, n, h]
    e = f_A @ f_B^T                               [b, m, n]
    beta  = softmax(e, axis=-1) @ B               [b, m, d]
    alpha = softmax(e, axis=-2)^T @ A             [b, n, d]
    returns (beta, alpha)

Shapes: b=4, m=n=4096, d=128, h=256. Scores e lie in ~[0.5, 8.3] so
exp() needs no max-subtraction: softmax = exp(e)/sum(exp(e)) directly,
and cross-shard softmax combines are plain sums of partials.

Sharding: 8 cores = (batch, m-half). Each core handles 2048 m-rows of
one batch against all 4096 n. beta is fully local (row softmax over n).
alpha needs a sum over m across the 2 cores of a batch: each core emits
unnormalized alpha^T partials + column-sum partials; the host adds the
two partials and divides (exact).

On-core dataflow (vs the 213us baseline which computed E twice, once
per orientation): E is computed ONCE (f32r, full PE rate at 512-moving),
exp'd on ACT to bf16 P (exact f32 rowsums via accum_out), and the
transposed orientation P^T is produced by the DMA xbar transpose engine
(dma_start_transpose, bf16, 14ns per 16x128 tile) — zero PE/ACT cost.
Both attention accumulations then run as bf16 matmuls (1.0 cycles/row,
same PE rate as f32r; rel err ~9e-4 vs the 2e-2 gate). Column sums for
the alpha softmax come from four independent quarter-chains on DVE
(one bf16 fold + exact f32 reduce per 4-chunk m-group, f32 partial
adds), so only one quarter-chain trails the last pair's final
transpose; both beta and alpha leave unnormalized (transposed) with
their softmax denominators and the host divides.

Loop structure: 8 n-strips of 512, processed in PAIRS of 1024 so each
m-chunk's P piece [128m, 1024n] needs only ONE xbar transpose per pair
(64 transpose DMAs total — more/smaller DMAs trip the 16-deep in-flight
DMA ring whose slot-recycle gates convoy the whole pipeline). Per strip,
per m-chunk: E piece [128m, 512n] (PE, 2 matmuls) -> exp -> P half
(+rowsum partial) -> alpha^T psum accumulation over the 16 chunks; in
odd strips the completed P piece is xbar-transposed into the pair's P^T
buffer [128n(8 blocks), 2048m]. Pair p's 32 beta matmuls (accumulating
beta^T [128d, 2048m] over all 32 n-blocks in 4 resident PSUM banks) and
its colsum folds are deferred to pair p+1 (one beta per chunk slot), by
which time the transposes have long landed. PSUM: 4 beta accumulators +
2 rotating alpha banks + 2 rotating E banks = 8.

Pair 0 runs with FUSED 1024-wide E/exp per chunk (the beta accumulator
banks aren't live yet, so they host double-width et tiles): one exp per
chunk-pair halves the per-instruction ACT overhead (185ns PSUM-read
bubble + 187ns accum read) where ACT would otherwise pace the strips.

Steady-state strips run at the PE floor (16 chunks x 4 matmuls x 213ns
= 13.6us vs ACT's 12.8us of exps): per-core PE = MLP 36.9k + E 131.1k
+ alpha 65.5k + beta 65.5k cycles ~= 125 us busy @2.4GHz; sim 140.4us
(89% PE busy; head ~6.5us DMA-init + MLP-phase partially DMA-paced
+ ~4.5us tail). ACT ~115 us, DMA pipe ~81 us, DVE ~70 us hide under it.
"""

import sys

import numpy as np

if "/opt/trn_rl_repo" not in sys.path:
    sys.path.insert(0, "/opt/trn_rl_repo")

import ml_dtypes  # noqa: E402

import concourse.bass as bass  # noqa: E402
import concourse.mybir as mybir  # noqa: E402
import concourse.tile as tile  # noqa: E402
from concourse import bacc  # noqa: E402

F32 = mybir.dt.float32
F32R = mybir.dt.float32r
BF16 = mybir.dt.bfloat16
EXP = mybir.ActivationFunctionType.Exp
COPY = mybir.ActivationFunctionType.Copy
RELU = mybir.ActivationFunctionType.Relu

D = 128      # model dim
H = 256      # hidden dim
M = 2048     # rows per core (half of 4096)
N = 4096     # full sequence
MC = M // 128   # m chunks per core (16)
NB = N // 128   # n blocks (32)
NS = N // 512   # n strips (8)

_CACHE = {}


def _mlp_transposed(nc, pools, xT, fT0, fT1, w1, w2, b1c, b2c, zero, seq):
    """fT = relu(W2^T @ relu(W1^T @ xT + b1) + b2) in transposed layout.

    xT: [128 d, seq] f32r;  fT0/fT1: [128, seq] f32r (h split in 2 blocks).
    h1 bias+relu runs on ACT, h2 bias+relu on DVE (splits the elementwise
    load so neither engine gates the PE).
    """
    mlp_ps, h1_pool = pools
    h1 = [h1_pool.tile([128, seq], F32R, tag=f"h1_{i}", name=f"h1_{i}")
          for i in range(2)]
    nchunks = seq // 512
    for s in range(nchunks):
        sl = bass.ts(s, 512)
        for i in range(2):
            ps = mlp_ps.tile([128, 512], F32, tag="mlp")
            nc.tensor.matmul(ps, w1[:, bass.ts(i, 128)], xT[:, sl],
                             start=True, stop=True)
            nc.scalar.activation(h1[i][:, sl], ps, RELU,
                                 bias=b1c[:, i:i + 1])
        for i, fT in enumerate((fT0, fT1)):
            if s == 0:
                continue
            ps = mlp_ps.tile([128, 512], F32, tag="mlp")
            sl_prev = bass.ts(s - 1, 512)
            for kh in range(2):
                nc.tensor.matmul(ps, w2[:, bass.ts(kh * 2 + i, 128)],
                                 h1[kh][:, sl_prev],
                                 start=(kh == 0), stop=(kh == 1))
            nc.vector.tensor_scalar(
                out=fT[:, sl_prev], in0=ps,
                scalar1=b2c[:, i:i + 1], scalar2=zero,
                op0=mybir.AluOpType.add, op1=mybir.AluOpType.max)
    sl = bass.ts(nchunks - 1, 512)
    for i, fT in enumerate((fT0, fT1)):
        ps = mlp_ps.tile([128, 512], F32, tag="mlp")
        for kh in range(2):
            nc.tensor.matmul(ps, w2[:, bass.ts(kh * 2 + i, 128)],
                             h1[kh][:, sl],
                             start=(kh == 0), stop=(kh == 1))
        nc.vector.tensor_scalar(
            out=fT[:, sl], in0=ps,
            scalar1=b2c[:, i:i + 1], scalar2=zero,
            op0=mybir.AluOpType.add, op1=mybir.AluOpType.max)


def _build():
    """Build + compile the per-core Bass program (same NEFF on all 8 cores)."""
    nc = bacc.Bacc(None, target_bir_lowering=False)

    # inputs (f32r tensors take plain fp32 host arrays)
    atb = nc.declare_dram_parameter("atb", [128, M], F32R, isOutput=False)
    btb = nc.declare_dram_parameter("btb", [128, N], F32R, isOutput=False)
    anr = nc.declare_dram_parameter("anr", [128, M], BF16, isOutput=False)
    bnr = nc.declare_dram_parameter("bnr", [128, N], BF16, isOutput=False)
    w1 = nc.declare_dram_parameter("w1", [128, H], F32R, isOutput=False)
    w2 = nc.declare_dram_parameter("w2", [128, 2 * H], F32R, isOutput=False)
    b1 = nc.declare_dram_parameter("b1", [128, 2], F32, isOutput=False)
    b2 = nc.declare_dram_parameter("b2", [128, 2], F32, isOutput=False)
    # outputs (beta leaves transposed + unnormalized: the host divides by
    # the rowsums and transposes, mirroring what it already does for alpha)
    betat_d = nc.declare_dram_parameter("betat", [128, M], F32, isOutput=True)
    rows_d = nc.declare_dram_parameter("rows", [128, MC * NS], F32,
                                       isOutput=True)
    alphat_d = nc.declare_dram_parameter("alphat", [128, N], F32, isOutput=True)
    cols_d = nc.declare_dram_parameter("cols", [128, NB], F32, isOutput=True)

    with tile.TileContext(nc) as tc, \
         tc.tile_pool(name="const", bufs=1) as const:
        # persistent SBUF tensors
        w1_sb = const.tile([128, H], F32R, tag="w1")
        w2_sb = const.tile([128, 2 * H], F32R, tag="w2")
        b1_sb = const.tile([128, 2], F32, tag="b1")
        b2_sb = const.tile([128, 2], F32, tag="b2")
        anr_sb = const.tile([128, M], BF16, tag="anr")
        bnr_sb = const.tile([128, N], BF16, tag="bnr")

        zero = const.tile([128, 1], F32, tag="zero")
        nc.vector.memset(zero, 0.0)
        # trigger the exp table-set load on ACT immediately (overlaps with
        # the input DMAs instead of stalling the first real exp)
        dummy = const.tile([128, 1], F32, tag="dummy")
        nc.scalar.activation(dummy, zero, EXP)

        fbt = [const.tile([128, N], F32R, tag=f"fbt{k}", name=f"fbt{k}")
               for k in range(2)]
        fat = [const.tile([128, M], F32R, tag=f"fat{k}", name=f"fat{k}")
               for k in range(2)]
        rows_sb = const.tile([128, MC * NS], F32, tag="rows")
        cols_sb = const.tile([128, NB], F32, tag="cols")
        betat_sb = const.tile([128, M], F32, tag="bt")

        # ---- phase 1: MLPs (atb/btb live in a pool that closes after) ----
        with tc.tile_pool(name="mlp_in", bufs=1) as mlp_in, \
             tc.tile_pool(name="mlp_ps", bufs=4, space="PSUM") as mlp_ps, \
             tc.tile_pool(name="h1", bufs=2) as h1_pool:
            atb_sb = mlp_in.tile([128, M], F32R, tag="atb")
            btb_sb = mlp_in.tile([128, N], F32R, tag="btb")
            # DMA issue is expensive and transfers drain through a mostly-
            # serial pipe: first operands first, ordered by first use.
            nc.gpsimd.dma_start(atb_sb[:, 0:512], atb[:, 0:512])
            nc.sync.dma_start(w1_sb, w1[:])
            nc.sync.dma_start(b1_sb, b1[:])
            nc.sync.dma_start(w2_sb, w2[:])
            nc.sync.dma_start(b2_sb, b2[:])
            nc.gpsimd.dma_start(atb_sb[:, 512:2048], atb[:, 512:2048])
            nc.gpsimd.dma_start(btb_sb[:, 0:1024], btb[:, 0:1024])
            nc.gpsimd.dma_start(btb_sb[:, 1024:2048], btb[:, 1024:2048])
            nc.gpsimd.dma_start(btb_sb[:, 2048:4096], btb[:, 2048:4096])
            nc.gpsimd.dma_start(anr_sb, anr[:])
            nc.gpsimd.dma_start(bnr_sb, bnr[:])

            # A's MLP first: it is half the size, and the first E strip is
            # gated by max(fat chunk 0, fbt chunk 0).
            pools = (mlp_ps, h1_pool)
            _mlp_transposed(nc, pools, atb_sb, fat[0], fat[1],
                            w1_sb, w2_sb, b1_sb, b2_sb, zero, M)
            _mlp_transposed(nc, pools, btb_sb, fbt[0], fbt[1],
                            w1_sb, w2_sb, b1_sb, b2_sb, zero, N)

        # ---- phase 2: fused E/exp/transpose/alpha/beta main loop ----
        # Strips are processed in PAIRS (1024 n): each m-chunk's P piece
        # spans both strips of the pair so ONE xbar transpose per
        # (chunk, pair) suffices — 64 transpose DMAs total instead of 128,
        # halving the HWDGE issue cost and the pressure on the 16-deep
        # in-flight DMA ring whose slot-recycle gates otherwise convoy
        # the whole pipeline.
        NP = NS // 2   # strip pairs (4)
        with tc.tile_pool(name="al_ps", bufs=2, space="PSUM") as al_pool, \
             tc.tile_pool(name="et_ps", bufs=2, space="PSUM") as et_pool, \
             tc.tile_pool(name="pp", bufs=1) as ppool, \
             tc.tile_pool(name="pt", bufs=2) as ptpool, \
             tc.tile_pool(name="fold", bufs=1) as foldpool, \
             tc.tile_pool(name="stage", bufs=2) as stage:
            # Pair 0 runs with FUSED 1024-wide E/exp per chunk: the beta
            # accumulators aren't live yet, so their 4 PSUM banks host
            # double-width et tiles instead; one exp per chunk-pair halves
            # the per-instruction ACT overhead (PSUM-read bubble + accum
            # read) while the rowsum partial still covers whole m-rows.
            nc.vector.memset(rows_sb, 0.0)
            et2_cm = tc.tile_pool(name="et2_ps", bufs=2, space="PSUM")
            et2_pool = et2_cm.__enter__()
            beta_ps = []

            def make_cols(p, pt_t):
                # column sums of pair p: two bf16 fold stages (DVE 2x
                # mode) then exact f32 reduce (P^T pair is
                # [128 n, 8 j, 2048 m]; fold errors are independent across
                # the surviving elements so the colsum error stays ~0.03%).
                # g1 depends only on m-chunks 0-7 (transposed by mid-strip)
                # so for the last pair half the chain overlaps the strip.
                def emit():
                    # four independent quarter-chains: each folds its own
                    # 4-chunk m-group (one bf16 fold stage) and reduces to
                    # an exact f32 [128, 8] partial; partials are summed in
                    # f32. Each quarter only needs its own 4 transposes, so
                    # for the last pair just one quarter-chain trails the
                    # final transpose.
                    rqs = []
                    for qi in range(4):
                        fq = foldpool.tile([128, 8, 256], BF16,
                                           tag=f"fq{qi % 2}",
                                           name=f"fq{qi}_{p}")
                        rq = foldpool.tile([128, 8], F32, tag=f"rq{qi}",
                                           name=f"rq{qi}_{p}")
                        base = qi * 512
                        nc.vector.tensor_tensor(
                            out=fq, in0=pt_t[:, :, base:base + 256],
                            in1=pt_t[:, :, base + 256:base + 512],
                            op=mybir.AluOpType.add)
                        nc.vector.tensor_reduce(
                            rq, fq, axis=mybir.AxisListType.X,
                            op=mybir.AluOpType.add)
                        rqs.append(rq)
                    cs = cols_sb[:, p * 8:(p + 1) * 8]
                    nc.vector.tensor_tensor(out=cs, in0=rqs[0], in1=rqs[1],
                                            op=mybir.AluOpType.add)
                    nc.vector.tensor_tensor(out=cs, in0=cs, in1=rqs[2],
                                            op=mybir.AluOpType.add)
                    nc.vector.tensor_tensor(out=cs, in0=cs, in1=rqs[3],
                                            op=mybir.AluOpType.add)
                return emit

            def make_beta(p, pt_t):
                # beta^T accumulation over pair p's 8 n-blocks, deferred
                # to pair p+1 (one matmul per chunk slot there).
                def emit(q, j):
                    nc.tensor.matmul(
                        beta_ps[q], bnr_sb[:, bass.ts(p * 8 + j, 128)],
                        pt_t[:, j, bass.ts(q, 512)],
                        start=(p == 0 and j == 0),
                        stop=(p == NP - 1 and j == 7))
                return [
                    (lambda q=q, j=j: emit(q, j))
                    for q in range(4) for j in range(8)
                ]

            pending = []      # previous pair's beta matmuls
            cols_pend = None  # previous pair's colsum emission

            # -- pair 0: fused 1024-wide chunks --
            pt_t = ptpool.tile([128, 8, 2048], BF16, tag="pt", name="pt0")
            p_tiles = [ppool.tile([128, 1024], BF16, tag=f"P{c}",
                                  name=f"P{c}_0") for c in range(MC)]
            al0 = al_pool.tile([128, 512], F32, tag="al", name="al0")
            al1 = al_pool.tile([128, 512], F32, tag="al", name="al1")
            for c in range(MC):
                et = et2_pool.tile([128, 1024], F32, tag="et2")
                for half in range(2):
                    for k in range(2):
                        nc.tensor.matmul(et[:, bass.ts(half, 512)],
                                         fat[k][:, bass.ts(c, 128)],
                                         fbt[k][:, bass.ts(half, 512)],
                                         start=(k == 0), stop=(k == 1))
                p_c = p_tiles[c]
                nc.scalar.activation(
                    p_c, et, EXP,
                    accum_out=rows_sb[:, c * NS:c * NS + 1])
                nc.tensor.matmul(al0, anr_sb[:, bass.ts(c, 128)],
                                 p_c[:, 0:512],
                                 start=(c == 0), stop=(c == MC - 1))
                nc.tensor.matmul(al1, anr_sb[:, bass.ts(c, 128)],
                                 p_c[:, 512:1024],
                                 start=(c == 0), stop=(c == MC - 1))
                nc.sync.dma_start_transpose(
                    pt_t[:, :, c * 128:(c + 1) * 128], p_c[:, :])
            a_sb = stage.tile([128, 1024], F32, tag="as", name="as0")
            nc.vector.tensor_copy(a_sb[:, 0:512], al0)
            nc.vector.tensor_copy(a_sb[:, 512:1024], al1)
            nc.gpsimd.dma_start(alphat_d[:, 0:1024], a_sb)
            cols_pend = make_cols(0, pt_t)
            pending = make_beta(0, pt_t)
            # double-width et banks free; open the beta accumulators
            et2_cm.__exit__(None, None, None)
            acc_cm = tc.tile_pool(name="acc_ps", bufs=1, space="PSUM")
            acc_pool = acc_cm.__enter__()
            beta_ps.extend(acc_pool.tile([128, 512], F32, tag=f"acc{q}",
                                         name=f"acc{q}") for q in range(4))

            # -- pairs 1-3: per-strip chunks --
            for s in range(2, NS):
                p, half = divmod(s, 2)
                nsl = bass.ts(s, 512)
                alpha_ps = al_pool.tile([128, 512], F32, tag="al",
                                        name=f"al{s}")
                if half == 0:
                    if cols_pend is not None:
                        cols_pend()
                        cols_pend = None
                    pt_t = ptpool.tile([128, 8, 2048], BF16, tag="pt",
                                       name=f"pt{p}")
                    p_tiles = [ppool.tile([128, 1024], BF16, tag=f"P{c}",
                                          name=f"P{c}_{p}")
                               for c in range(MC)]
                for c in range(MC):
                    et = et_pool.tile([128, 512], F32, tag="et")
                    for k in range(2):
                        nc.tensor.matmul(et, fat[k][:, bass.ts(c, 128)],
                                         fbt[k][:, nsl],
                                         start=(k == 0), stop=(k == 1))
                    if pending:
                        pending.pop(0)()
                    p_c = p_tiles[c]
                    nc.scalar.activation(
                        p_c[:, bass.ts(half, 512)], et,
                        EXP, accum_out=rows_sb[:, c * NS + s:c * NS + s + 1])
                    nc.tensor.matmul(alpha_ps, anr_sb[:, bass.ts(c, 128)],
                                     p_c[:, bass.ts(half, 512)],
                                     start=(c == 0), stop=(c == MC - 1))
                    if half == 1:
                        # xbar transpose of the completed [128, 1024] piece:
                        # pt_t[p_, j, c*128+q] = p_c[q, j*128+p_]
                        nc.sync.dma_start_transpose(
                            pt_t[:, :, c * 128:(c + 1) * 128], p_c[:, :])
                # evacuate alpha^T strip (unnormalized; host divides)
                a_sb = (stage.tile([128, 1024], F32, tag="as",
                                   name=f"as{p}") if half == 0
                        else a_sb)
                nc.vector.tensor_copy(a_sb[:, bass.ts(half, 512)], alpha_ps)
                if half == 1:
                    nc.gpsimd.dma_start(
                        alphat_d[:, p * 1024:(p + 1) * 1024], a_sb)
                    cols_pend = make_cols(p, pt_t)
                    pending = make_beta(p, pt_t)
            for fn in pending:
                fn()

            # -- finalize: evacuate beta^T psum on ACT (idle once the exps
            # are done) and ship it + the raw rowsum partials; the DVE
            # tail is just the last pair's colsum folds --
            for q in range(4):
                nc.scalar.activation(betat_sb[:, bass.ts(q, 512)],
                                     beta_ps[q], COPY)
                nc.sync.dma_start(betat_d[:, bass.ts(q, 512)],
                                  betat_sb[:, bass.ts(q, 512)])
            nc.sync.dma_start(rows_d[:], rows_sb)

            cols_pend()
            nc.sync.dma_start(cols_d[:], cols_sb)
            acc_cm.__exit__(None, None, None)

    nc.compile()
    return nc


def _get_nc():
    if "nc" not in _CACHE:
        _CACHE["nc"] = _build()
    return _CACHE["nc"]


def _get_runner():
    """Jitted 8-core shard_map executor built once (mirrors
    bass2jax.run_bass_via_pjrt, but cacheable across calls)."""
    if "runner" in _CACHE:
        return _CACHE["runner"]
    import jax
    from jax.sharding import Mesh, PartitionSpec
    from jax.experimental.shard_map import shard_map
    import concourse.mybir as mb
    from concourse.bass2jax import (
        _bass_exec_p, install_neuronx_cc_hook, partition_id_tensor)

    nc = _get_nc()
    install_neuronx_cc_hook()

    in_names, out_names, out_avals = [], [], []
    partition_name = (nc.partition_id_tensor.name
                      if nc.partition_id_tensor else None)
    for alloc in nc.m.functions[0].allocations:
        if not isinstance(alloc, mb.MemoryLocationSet):
            continue
        name = alloc.memorylocations[0].name
        if alloc.kind == "ExternalInput":
            if name != partition_name:
                in_names.append(name)
        elif alloc.kind == "ExternalOutput":
            out_names.append(name)
            out_avals.append(jax.core.ShapedArray(
                tuple(alloc.tensor_shape), mb.dt.np(alloc.dtype)))
    n_params = len(in_names)
    zero_outs = [np.zeros((8 * a.shape[0], *a.shape[1:]), a.dtype)
                 for a in out_avals]
    all_in_names = in_names + out_names
    if partition_name is not None:
        all_in_names = all_in_names + [partition_name]

    def _body(*args):
        operands = list(args)
        if partition_name is not None:
            operands.append(partition_id_tensor())
        return tuple(_bass_exec_p.bind(
            *operands,
            out_avals=tuple(out_avals),
            in_names=tuple(all_in_names),
            out_names=tuple(out_names),
            lowering_input_output_aliases=(),
            sim_require_finite=True,
            sim_require_nnan=True,
            nc=nc,
        ))

    devices = jax.devices()[:8]
    mesh = Mesh(np.asarray(devices), ("core",))
    nin = n_params + len(out_names)
    sharded = jax.jit(shard_map(
        _body, mesh=mesh,
        in_specs=(PartitionSpec("core"),) * nin,
        out_specs=(PartitionSpec("core"),) * len(out_names),
        check_rep=False))
    zeros_dev = [jax.device_put(z) for z in zero_outs]
    _CACHE["runner"] = (sharded, in_names, out_names, out_avals, zeros_dev)
    return _CACHE["runner"]


def run_cores(in_maps):
    """Run the 8-core program; returns list of per-core output dicts."""
    sharded, in_names, out_names, out_avals, zeros_dev = _get_runner()
    concat_in = [np.concatenate([m[name] for m in in_maps], axis=0)
                 for name in in_names]
    out_arrs = sharded(*concat_in, *zeros_dev)
    out_arrs = [np.asarray(o) for o in out_arrs]
    return [
        {name: out_arrs[i].reshape(8, *out_avals[i].shape)[c]
         for i, name in enumerate(out_names)}
        for c in range(8)
    ]


def build_in_maps(A, B, W1, b1, W2, b2):
    A = np.ascontiguousarray(np.asarray(A, dtype=np.float32))
    B = np.ascontiguousarray(np.asarray(B, dtype=np.float32))
    W1 = np.asarray(W1, dtype=np.float32)
    b1 = np.asarray(b1, dtype=np.float32)
    W2 = np.asarray(W2, dtype=np.float32)
    b2 = np.asarray(b2, dtype=np.float32)
    nbatch, seq, d = A.shape
    assert (nbatch, seq, d) == (4, N, D), (nbatch, seq, d)

    w1r = np.ascontiguousarray(W1)                                # [128, 256]
    w2r = np.ascontiguousarray(
        W2.reshape(2, 128, 2, 128).transpose(1, 0, 2, 3).reshape(128, 512))
    b1c = np.ascontiguousarray(b1.reshape(2, 128).T)              # [128, 2]
    b2c = np.ascontiguousarray(b2.reshape(2, 128).T)

    in_maps = []
    for core in range(8):
        b_i, half = divmod(core, 2)
        Ah = A[b_i, half * M:(half + 1) * M]                      # [2048, 128]
        Bf = B[b_i]                                               # [4096, 128]
        in_maps.append({
            "atb": np.ascontiguousarray(Ah.T),
            "btb": np.ascontiguousarray(Bf.T),
            "anr": np.ascontiguousarray(
                Ah.reshape(MC, 128, 128).transpose(1, 0, 2).reshape(128, M)
            ).astype(ml_dtypes.bfloat16),
            "bnr": np.ascontiguousarray(
                Bf.reshape(NB, 128, 128).transpose(1, 0, 2).reshape(128, N)
            ).astype(ml_dtypes.bfloat16),
            "w1": w1r, "w2": w2r, "b1": b1c, "b2": b2c,
        })
    return in_maps


def kernel(A, B, W1, b1, W2, b2):
    in_maps = build_in_maps(A, B, W1, b1, W2, b2)
    results = run_cores(in_maps)

    beta = np.empty((4, N, D), dtype=np.float32)
    alpha = np.empty((4, N, D), dtype=np.float32)
    for b_i in range(4):
        r0 = results[2 * b_i]
        r1 = results[2 * b_i + 1]
        for half, r in ((0, r0), (1, r1)):
            # rows[p, c*8+s] are per-strip rowsum partials for m = c*128+p
            rowv = r["rows"].reshape(128, MC, NS).sum(2).T.reshape(1, M)
            beta[b_i, half * M:(half + 1) * M] = (r["betat"] / rowv).T
        num = r0["alphat"] + r1["alphat"]                          # [128, 4096]
        csum = r0["cols"] + r1["cols"]                             # [128, 32]
        # csum[p, j] corresponds to n = j*128 + p
        alpha[b_i] = (num / csum.T.reshape(1, N)).T.reshape(N, D)
    return beta, alpha


if __name__ == "__main__":
    rng = np.random.default_rng(0)
    A = rng.standard_normal((4, N, D)).astype(np.float32)
    B = rng.standard_normal((4, N, D)).astype(np.float32)
    s1, s2 = 1.0 / np.sqrt(D), 1.0 / np.sqrt(H)
    W1 = rng.uniform(-s1, s1, (D, H)).astype(np.float32)
    b1 = rng.uniform(-s1, s1, H).astype(np.float32)
    W2 = rng.uniform(-s2, s2, (H, H)).astype(np.float32)
    b2 = rng.uniform(-s2, s2, H).astype(np.float32)
    beta, alpha = kernel(A=A, B=B, W1=W1, b1=b1, W2=W2, b2=b2)
    print("beta", beta.shape, "alpha", alpha.shape)
